# revision 6
# baseline (speedup 1.0000x reference)
"""Trainium2 kernel for nn_AnimaMLP (moe_routing, 8 NeuronCores).

Data-parallel over tokens: 8192 tokens are sharded 1024/core, router +
all 8 experts run per-core on the token shard (weights replicated), no
collectives. Expert compute in bf16 (fp32 PSUM accumulation), router in
fp32 so the top-5 selection matches the fp32 reference.

Self-contained: shapes hardcoded, no sibling imports.
"""
import math

import numpy as np

import concourse.bacc as bacc
import concourse.bass as bass
import concourse.mybir as mybir
import concourse.tile as tile
from concourse import masks
from concourse.bass_utils import run_bass_kernel_spmd

# Problem shape
B, T, D, I, E = 4, 2048, 2048, 1024, 8
N_CORES = 8
TOKENS = B * T              # 8192
TC = TOKENS // N_CORES      # 1024 tokens per core
TEMP = math.e
N_ACT = 5                   # top-k
N_CAMP_A = 4                # experts 0..3 positive, 4..7 negative

P = 128
DC = D // P                 # 16 d-chunks
IC = I // P                 # 8 i-chunks
TBLK = 512                  # tokens per block
NBLK = TC // TBLK           # 2 blocks per core
TPB = TBLK // P             # 4 tok128-chunks per block

F32 = mybir.dt.float32
BF16 = mybir.dt.bfloat16

_cached = {}


def build(tokens_per_core=TC, n_experts=E):
    nc = bacc.Bacc("TRN2", target_bir_lowering=False, debug=False,
                   num_devices=N_CORES)
    tc_tokens = tokens_per_core
    nblk = tc_tokens // TBLK
    n_tc = tc_tokens // P  # tok128 chunks per core

    x_d = nc.dram_tensor("x", [tc_tokens, D], F32, kind="ExternalInput").ap()
    wr_d = nc.dram_tensor("wr", [D, E], F32, kind="ExternalInput").ap()
    wg_d = nc.dram_tensor("wg", [E, D, I], F32, kind="ExternalInput").ap()
    wu_d = nc.dram_tensor("wu", [E, D, I], F32, kind="ExternalInput").ap()
    wd_d = nc.dram_tensor("wd", [E, I, D], F32, kind="ExternalInput").ap()
    out_d = nc.dram_tensor("out", [tc_tokens, D], F32, kind="ExternalOutput").ap()
    ss_d = nc.dram_tensor("ss", [P, n_tc], F32, kind="ExternalOutput").ap()

    au = mybir.AluOpType

    with tile.TileContext(nc) as tcx:
        import contextlib
        with contextlib.ExitStack() as top:
            const = top.enter_context(tcx.tile_pool(name="const", bufs=1))
            wpool = top.enter_context(tcx.tile_pool(name="wpool", bufs=1))
            xtb_pool = top.enter_context(tcx.tile_pool(name="xtb", bufs=1))
            acc_pool = top.enter_context(tcx.tile_pool(name="accp", bufs=1))

            ident = const.tile([P, P], F32)
            masks.make_identity(nc, ident[:])
            wr_sb = const.tile([P, DC, E], F32)
            nc.sync.dma_start(wr_sb[:], wr_d.rearrange("(do p) e -> p do e", p=P))
            sign = const.tile([P, E], F32)
            nc.vector.memset(sign[:, 0:N_CAMP_A], 1.0)
            nc.vector.memset(sign[:, N_CAMP_A:E], -1.0)
            ss_sb = const.tile([P, n_tc], F32)

            xT_bf = [[None] * DC for _ in range(nblk)]
            w_signed = [None] * n_tc

            # ---- stage 0: transpose x (PE), router in fp32 ----
            with contextlib.ExitStack() as s0:
                xin_pool = s0.enter_context(tcx.tile_pool(name="xin", bufs=1))
                tp_psum = s0.enter_context(
                    tcx.tile_pool(name="tpps", bufs=4, space="PSUM"))
                xtf_pool = s0.enter_context(tcx.tile_pool(name="xtf", bufs=1))
                r_psum = s0.enter_context(
                    tcx.tile_pool(name="rps", bufs=2, space="PSUM"))
                r_sb = s0.enter_context(tcx.tile_pool(name="rsb", bufs=3))

                xT_f32 = [[None] * DC for _ in range(nblk)]
                for b in range(nblk):
                    xin = []
                    for t in range(TPB):
                        xt = xin_pool.tile([P, D], F32, name=f"xin{b}_{t}", tag=f"xin{t}")
                        row0 = (b * TPB + t) * P
                        nc.sync.dma_start(xt[:], x_d[row0:row0 + P, :])
                        xin.append(xt)
                    for do in range(DC):
                        ps = tp_psum.tile([P, TBLK], F32, name=f"tp{b}_{do}",
                                          tag="tp")
                        for t in range(TPB):
                            nc.tensor.transpose(
                                ps[:, t * P:(t + 1) * P],
                                xin[t][:, do * P:(do + 1) * P], ident[:])
                        xf = xtf_pool.tile([P, TBLK], F32, name=f"xtf{b}_{do}")
                        nc.vector.tensor_copy(xf[:], ps[:])
                        xT_f32[b][do] = xf
                        xb = xtb_pool.tile([P, TBLK], BF16, name=f"xtb{b}_{do}")
                        nc.scalar.copy(xb[:], xf[:])
                        xT_bf[b][do] = xb

                # router per tok128 chunk
                for tci in range(n_tc):
                    b, t = tci // TPB, tci % TPB
                    ps_r = r_psum.tile([P, E], F32, name=f"rps{tci}", tag="rps")
                    for do in range(DC):
                        nc.tensor.matmul(
                            ps_r[:], xT_f32[b][do][:, t * P:(t + 1) * P],
                            wr_sb[:, do, :],
                            start=(do == 0), stop=(do == DC - 1))
                    s = r_sb.tile([P, E], F32, name=f"rs{tci}", tag="rs")
                    nc.vector.tensor_copy(s[:], ps_r[:])
                    m8 = r_sb.tile([P, 8], F32, name=f"rm8{tci}", tag="rm8")
                    nc.vector.max(m8[:], s[:])
                    bias = r_sb.tile([P, 1], F32, name=f"rb{tci}", tag="rb")
                    nc.scalar.mul(bias[:], m8[:, 0:1], -1.0 / TEMP)
                    p_un = r_sb.tile([P, E], F32, name=f"rp{tci}", tag="rp")
                    ssum = r_sb.tile([P, 1], F32, name=f"rsum{tci}", tag="rsum")
                    nc.scalar.activation(
                        p_un[:], s[:], mybir.ActivationFunctionType.Exp,
                        bias=bias[:, 0:1], scale=1.0 / TEMP,
                        accum_out=ssum[:, 0:1])
                    recip = r_sb.tile([P, 1], F32, name=f"rrec{tci}", tag="rrec")
                    nc.vector.reciprocal(recip[:], ssum[:])
                    probs = r_sb.tile([P, E], F32, name=f"rpr{tci}", tag="rpr")
                    nc.vector.tensor_scalar_mul(probs[:], p_un[:], recip[:, 0:1])
                    mask = r_sb.tile([P, E], F32, name=f"rmk{tci}", tag="rmk")
                    nc.vector.tensor_scalar(
                        mask[:], s[:], m8[:, N_ACT - 1:N_ACT], None,
                        op0=au.is_ge)
                    pm = r_sb.tile([P, E], F32, name=f"rpm{tci}", tag="rpm")
                    msum = r_sb.tile([P, 1], F32, name=f"rms{tci}", tag="rms")
                    nc.vector.scalar_tensor_tensor(
                        pm[:], probs[:], 1.0, mask[:],
                        op0=au.mult, op1=au.mult, accum_out=msum[:, 0:1])
                    dn = r_sb.tile([P, 1], F32, name=f"rdn{tci}", tag="rdn")
                    nc.vector.tensor_scalar_add(dn[:], msum[:], 1e-8)
                    rec2 = r_sb.tile([P, 1], F32, name=f"rr2{tci}", tag="rr2")
                    nc.vector.reciprocal(rec2[:], dn[:])
                    wsg = wpool.tile([P, E], F32, name=f"wsg{tci}")
                    nc.vector.scalar_tensor_tensor(
                        wsg[:], pm[:], rec2[:, 0:1], sign[:],
                        op0=au.mult, op1=au.mult)
                    w_signed[tci] = wsg

            # ---- main loop: experts ----
            with contextlib.ExitStack() as sm:
                wg_pool = sm.enter_context(tcx.tile_pool(name="wgp", bufs=1))
                wu_pool = sm.enter_context(tcx.tile_pool(name="wup", bufs=1))
                wd_pool = sm.enter_context(tcx.tile_pool(name="wdp", bufs=1))
                h_pool = sm.enter_context(tcx.tile_pool(name="hp", bufs=IC + 2))
                sg_pool = sm.enter_context(tcx.tile_pool(name="sgp", bufs=4))
                gu_psum = sm.enter_context(
                    tcx.tile_pool(name="gups", bufs=2, space="PSUM"))
                eo_psum = sm.enter_context(
                    tcx.tile_pool(name="eops", bufs=2, space="PSUM"))
                dummy = sm.enter_context(tcx.tile_pool(name="dummyp", bufs=1))

                dmy = dummy.tile([P, D], F32)

                for b in range(nblk):
                    acc = [acc_pool.tile([P, D], F32, name=f"acc{b}_{t}",
                                         tag=f"acc{t}")
                           for t in range(TPB)]
                    for e in range(n_experts):
                        wg_sb = wg_pool.tile([P, DC, I], BF16,
                                             name=f"wg{b}_{e}", tag="wg")
                        nc.gpsimd.dma_start(
                            out=wg_sb[:],
                            in_=wg_d[e].rearrange("(do p) i -> p do i", p=P))
                        wu_sb = wu_pool.tile([P, DC, I], BF16,
                                             name=f"wu{b}_{e}", tag="wu")
                        nc.gpsimd.dma_start(
                            out=wu_sb[:],
                            in_=wu_d[e].rearrange("(do p) i -> p do i", p=P))

                        h_tiles = []
                        for i in range(IC):
                            g_ps = gu_psum.tile([P, TBLK], F32,
                                                name=f"g{b}_{e}_{i}", tag="gps")
                            for do in range(DC):
                                nc.tensor.matmul(
                                    g_ps[:],
                                    wg_sb[:, do, i * P:(i + 1) * P],
                                    xT_bf[b][do][:],
                                    start=(do == 0), stop=(do == DC - 1))
                            u_ps = gu_psum.tile([P, TBLK], F32,
                                                name=f"u{b}_{e}_{i}", tag="ups")
                            for do in range(DC):
                                nc.tensor.matmul(
                                    u_ps[:],
                                    wu_sb[:, do, i * P:(i + 1) * P],
                                    xT_bf[b][do][:],
                                    start=(do == 0), stop=(do == DC - 1))
                            sg = sg_pool.tile([P, TBLK], F32,
                                              name=f"sg{b}_{e}_{i}", tag="sg")
                            nc.scalar.activation(
                                sg[:], g_ps[:],
                                mybir.ActivationFunctionType.Sigmoid)
                            t2 = sg_pool.tile([P, TBLK], F32,
                                              name=f"t2{b}_{e}_{i}", tag="t2")
                            nc.vector.tensor_mul(t2[:], sg[:], g_ps[:])
                            h_sb = h_pool.tile([P, TBLK], BF16,
                                               name=f"h{b}_{e}_{i}", tag="h")
                            nc.vector.tensor_mul(h_sb[:], t2[:], u_ps[:])
                            h_tiles.append(h_sb)

                        wd_tiles = []
                        for i in range(IC):
                            wd_sb = wd_pool.tile([P, D], BF16,
                                                 name=f"wd{b}_{e}_{i}", tag=f"wd{i}")
                            nc.gpsimd.dma_start(
                                out=wd_sb[:],
                                in_=wd_d[e, i * P:(i + 1) * P, :])
                            wd_tiles.append(wd_sb)

                        for t in range(TPB):
                            tci = b * TPB + t
                            wcol = w_signed[tci][:, e:e + 1]
                            for half in range(2):
                                eo_ps = eo_psum.tile(
                                    [P, D // 2], F32,
                                    name=f"eo{b}_{e}_{t}_{half}", tag="eo")
                                for n in range(2):
                                    nsl = slice(n * TBLK, (n + 1) * TBLK)
                                    for i in range(IC):
                                        nc.tensor.matmul(
                                            eo_ps[:, nsl],
                                            h_tiles[i][:, t * P:(t + 1) * P],
                                            wd_tiles[i][:,
                                                        half * (D // 2)
                                                        + n * TBLK:
                                                        half * (D // 2)
                                                        + (n + 1) * TBLK],
                                            start=(i == 0), stop=(i == IC - 1))
                                acc_sl = acc[t][:, half * (D // 2):
                                                (half + 1) * (D // 2)]
                                if e == 0:
                                    nc.vector.tensor_scalar_mul(
                                        acc_sl, eo_ps[:], wcol)
                                else:
                                    nc.vector.scalar_tensor_tensor(
                                        acc_sl, eo_ps[:], wcol, acc_sl,
                                        op0=au.mult, op1=au.add)

                    for t in range(TPB):
                        tci = b * TPB + t
                        nc.sync.dma_start(
                            out_d[tci * P:(tci + 1) * P, :], acc[t][:])
                        nc.scalar.activation(
                            dmy[:], acc[t][:],
                            mybir.ActivationFunctionType.Square,
                            accum_out=ss_sb[:, tci:tci + 1])

            nc.sync.dma_start(ss_d[:], ss_sb[:])

    nc.compile()
    return nc


def _get_nc():
    if "nc" not in _cached:
        _cached["nc"] = build()
    return _cached["nc"]


def kernel(x, Wr, Wg, Wu, Wd, _trace=False):
    x = np.ascontiguousarray(np.asarray(x, dtype=np.float32))
    Wr = np.ascontiguousarray(np.asarray(Wr, dtype=np.float32))
    Wg = np.ascontiguousarray(np.asarray(Wg, dtype=np.float32))
    Wu = np.ascontiguousarray(np.asarray(Wu, dtype=np.float32))
    Wd = np.ascontiguousarray(np.asarray(Wd, dtype=np.float32))

    x2 = x.reshape(TOKENS, D)
    nc = _get_nc()
    in_maps = []
    for c in range(N_CORES):
        in_maps.append({
            "x": x2[c * TC:(c + 1) * TC],
            "wr": Wr, "wg": Wg, "wu": Wu, "wd": Wd,
        })
    res = run_bass_kernel_spmd(nc, in_maps, core_ids=list(range(N_CORES)),
                               trace=_trace)
    out = np.concatenate([res.results[c]["out"] for c in range(N_CORES)],
                         axis=0).reshape(B, T, D)
    ss = sum(float(res.results[c]["ss"].sum(dtype=np.float64))
             for c in range(N_CORES))
    tension = np.float32(ss / (TOKENS * D))
    if _trace:
        kernel.last_results = res
    return out, tension


# revision 7
# speedup vs baseline: 1.0159x; 1.0159x over previous
"""Trainium2 kernel for nn_AnimaMLP (moe_routing, 8 NeuronCores).

Data-parallel over tokens: 8192 tokens are sharded 1024/core, router +
all 8 experts run per-core on the token shard (weights replicated), no
collectives. Expert compute in bf16 (fp32 PSUM accumulation), router in
fp32 so the top-5 selection matches the fp32 reference.

Self-contained: shapes hardcoded, no sibling imports.
"""
import math

import numpy as np

import concourse.bacc as bacc
import concourse.bass as bass
import concourse.mybir as mybir
import concourse.tile as tile
from concourse import masks
from concourse.bass_utils import run_bass_kernel_spmd

# Problem shape
B, T, D, I, E = 4, 2048, 2048, 1024, 8
N_CORES = 8
TOKENS = B * T              # 8192
TC = TOKENS // N_CORES      # 1024 tokens per core
TEMP = math.e
N_ACT = 5                   # top-k
N_CAMP_A = 4                # experts 0..3 positive, 4..7 negative

P = 128
DC = D // P                 # 16 d-chunks
IC = I // P                 # 8 i-chunks
TBLK = 512                  # tokens per block
NBLK = TC // TBLK           # 2 blocks per core
TPB = TBLK // P             # 4 tok128-chunks per block

F32 = mybir.dt.float32
BF16 = mybir.dt.bfloat16

_cached = {}


def build(tokens_per_core=TC, n_experts=E):
    nc = bacc.Bacc("TRN2", target_bir_lowering=False, debug=False,
                   num_devices=N_CORES)
    tc_tokens = tokens_per_core
    nblk = tc_tokens // TBLK
    n_tc = tc_tokens // P  # tok128 chunks per core

    x_d = nc.dram_tensor("x", [tc_tokens, D], F32, kind="ExternalInput").ap()
    wr_d = nc.dram_tensor("wr", [D, E], F32, kind="ExternalInput").ap()
    wg_d = nc.dram_tensor("wg", [E, D, I], F32, kind="ExternalInput").ap()
    wu_d = nc.dram_tensor("wu", [E, D, I], F32, kind="ExternalInput").ap()
    wd_d = nc.dram_tensor("wd", [E, I, D], F32, kind="ExternalInput").ap()
    out_d = nc.dram_tensor("out", [tc_tokens, D], F32, kind="ExternalOutput").ap()
    ss_d = nc.dram_tensor("ss", [P, n_tc], F32, kind="ExternalOutput").ap()

    au = mybir.AluOpType

    with tile.TileContext(nc) as tcx:
        import contextlib
        with contextlib.ExitStack() as top:
            const = top.enter_context(tcx.tile_pool(name="const", bufs=1))
            wpool = top.enter_context(tcx.tile_pool(name="wpool", bufs=1))
            xtb_pool = top.enter_context(tcx.tile_pool(name="xtb", bufs=1))
            acc_pool = top.enter_context(tcx.tile_pool(name="accp", bufs=1))

            ident = const.tile([P, P], F32)
            masks.make_identity(nc, ident[:])
            wr_sb = const.tile([P, DC, E], F32)
            nc.sync.dma_start(wr_sb[:], wr_d.rearrange("(do p) e -> p do e", p=P))
            sign = const.tile([P, E], F32)
            nc.vector.memset(sign[:, 0:N_CAMP_A], 1.0)
            nc.vector.memset(sign[:, N_CAMP_A:E], -1.0)
            ss_sb = const.tile([P, n_tc], F32)

            xT_bf = [[None] * DC for _ in range(nblk)]
            w_signed = [None] * n_tc

            # ---- stage 0: transpose x (PE), router in fp32 ----
            with contextlib.ExitStack() as s0:
                xin_pool = s0.enter_context(tcx.tile_pool(name="xin", bufs=1))
                tp_psum = s0.enter_context(
                    tcx.tile_pool(name="tpps", bufs=4, space="PSUM"))
                xtf_pool = s0.enter_context(tcx.tile_pool(name="xtf", bufs=1))
                r_psum = s0.enter_context(
                    tcx.tile_pool(name="rps", bufs=2, space="PSUM"))
                r_sb = s0.enter_context(tcx.tile_pool(name="rsb", bufs=3))

                xT_f32 = [[None] * DC for _ in range(nblk)]
                for b in range(nblk):
                    xin = []
                    for t in range(TPB):
                        xt = xin_pool.tile([P, D], F32, name=f"xin{b}_{t}", tag=f"xin{t}")
                        row0 = (b * TPB + t) * P
                        nc.sync.dma_start(xt[:], x_d[row0:row0 + P, :])
                        xin.append(xt)
                    for do in range(DC):
                        ps = tp_psum.tile([P, TBLK], F32, name=f"tp{b}_{do}",
                                          tag="tp")
                        for t in range(TPB):
                            nc.tensor.transpose(
                                ps[:, t * P:(t + 1) * P],
                                xin[t][:, do * P:(do + 1) * P], ident[:])
                        xf = xtf_pool.tile([P, TBLK], F32, name=f"xtf{b}_{do}")
                        nc.vector.tensor_copy(xf[:], ps[:])
                        xT_f32[b][do] = xf
                        xb = xtb_pool.tile([P, TBLK], BF16, name=f"xtb{b}_{do}")
                        nc.scalar.copy(xb[:], xf[:])
                        xT_bf[b][do] = xb

                # router per tok128 chunk
                for tci in range(n_tc):
                    b, t = tci // TPB, tci % TPB
                    ps_r = r_psum.tile([P, E], F32, name=f"rps{tci}", tag="rps")
                    for do in range(DC):
                        nc.tensor.matmul(
                            ps_r[:], xT_f32[b][do][:, t * P:(t + 1) * P],
                            wr_sb[:, do, :],
                            start=(do == 0), stop=(do == DC - 1))
                    s = r_sb.tile([P, E], F32, name=f"rs{tci}", tag="rs")
                    nc.vector.tensor_copy(s[:], ps_r[:])
                    m8 = r_sb.tile([P, 8], F32, name=f"rm8{tci}", tag="rm8")
                    nc.vector.max(m8[:], s[:])
                    bias = r_sb.tile([P, 1], F32, name=f"rb{tci}", tag="rb")
                    nc.scalar.mul(bias[:], m8[:, 0:1], -1.0 / TEMP)
                    p_un = r_sb.tile([P, E], F32, name=f"rp{tci}", tag="rp")
                    ssum = r_sb.tile([P, 1], F32, name=f"rsum{tci}", tag="rsum")
                    nc.scalar.activation(
                        p_un[:], s[:], mybir.ActivationFunctionType.Exp,
                        bias=bias[:, 0:1], scale=1.0 / TEMP,
                        accum_out=ssum[:, 0:1])
                    recip = r_sb.tile([P, 1], F32, name=f"rrec{tci}", tag="rrec")
                    nc.vector.reciprocal(recip[:], ssum[:])
                    probs = r_sb.tile([P, E], F32, name=f"rpr{tci}", tag="rpr")
                    nc.vector.tensor_scalar_mul(probs[:], p_un[:], recip[:, 0:1])
                    mask = r_sb.tile([P, E], F32, name=f"rmk{tci}", tag="rmk")
                    nc.vector.tensor_scalar(
                        mask[:], s[:], m8[:, N_ACT - 1:N_ACT], None,
                        op0=au.is_ge)
                    pm = r_sb.tile([P, E], F32, name=f"rpm{tci}", tag="rpm")
                    msum = r_sb.tile([P, 1], F32, name=f"rms{tci}", tag="rms")
                    nc.vector.scalar_tensor_tensor(
                        pm[:], probs[:], 1.0, mask[:],
                        op0=au.mult, op1=au.mult, accum_out=msum[:, 0:1])
                    dn = r_sb.tile([P, 1], F32, name=f"rdn{tci}", tag="rdn")
                    nc.vector.tensor_scalar_add(dn[:], msum[:], 1e-8)
                    rec2 = r_sb.tile([P, 1], F32, name=f"rr2{tci}", tag="rr2")
                    nc.vector.reciprocal(rec2[:], dn[:])
                    wsg = wpool.tile([P, E], F32, name=f"wsg{tci}")
                    nc.vector.scalar_tensor_tensor(
                        wsg[:], pm[:], rec2[:, 0:1], sign[:],
                        op0=au.mult, op1=au.mult)
                    w_signed[tci] = wsg

            # ---- main loop: experts ----
            with contextlib.ExitStack() as sm:
                wg_pool = sm.enter_context(tcx.tile_pool(name="wgp", bufs=1))
                wu_pool = sm.enter_context(tcx.tile_pool(name="wup", bufs=1))
                wd_pool = sm.enter_context(tcx.tile_pool(name="wdp", bufs=1))
                h_pool = sm.enter_context(tcx.tile_pool(name="hp", bufs=IC + 2))
                sg_pool = sm.enter_context(tcx.tile_pool(name="sgp", bufs=4))
                gu_psum = sm.enter_context(
                    tcx.tile_pool(name="gups", bufs=2, space="PSUM"))
                eo_psum = sm.enter_context(
                    tcx.tile_pool(name="eops", bufs=2, space="PSUM"))
                dummy = sm.enter_context(tcx.tile_pool(name="dummyp", bufs=1))

                dmy = dummy.tile([P, D], F32)

                for b in range(nblk):
                    acc = [acc_pool.tile([P, D], F32, name=f"acc{b}_{t}",
                                         tag=f"acc{t}")
                           for t in range(TPB)]
                    for e in range(n_experts):
                        # quarter-tiles (2 i-chunks each) so the next
                        # expert's loads overlap this expert's matmuls
                        IQ = I // 4
                        wg_q, wu_q = [], []
                        for q in range(4):
                            isl = slice(q * IQ, (q + 1) * IQ)
                            wgt = wg_pool.tile([P, DC, IQ], BF16,
                                               name=f"wg{b}_{e}_{q}",
                                               tag=f"wgq{q}")
                            nc.gpsimd.dma_start(
                                out=wgt[:],
                                in_=wg_d[e][:, isl].rearrange(
                                    "(do p) i -> p do i", p=P))
                            wg_q.append(wgt)
                            wut = wu_pool.tile([P, DC, IQ], BF16,
                                               name=f"wu{b}_{e}_{q}",
                                               tag=f"wuq{q}")
                            nc.gpsimd.dma_start(
                                out=wut[:],
                                in_=wu_d[e][:, isl].rearrange(
                                    "(do p) i -> p do i", p=P))
                            wu_q.append(wut)

                        h_tiles = []
                        for i in range(IC):
                            q, iq = i // 2, i % 2
                            g_ps = gu_psum.tile([P, TBLK], F32,
                                                name=f"g{b}_{e}_{i}", tag="gps")
                            for do in range(DC):
                                nc.tensor.matmul(
                                    g_ps[:],
                                    wg_q[q][:, do, iq * P:(iq + 1) * P],
                                    xT_bf[b][do][:],
                                    start=(do == 0), stop=(do == DC - 1))
                            u_ps = gu_psum.tile([P, TBLK], F32,
                                                name=f"u{b}_{e}_{i}", tag="ups")
                            for do in range(DC):
                                nc.tensor.matmul(
                                    u_ps[:],
                                    wu_q[q][:, do, iq * P:(iq + 1) * P],
                                    xT_bf[b][do][:],
                                    start=(do == 0), stop=(do == DC - 1))
                            sg = sg_pool.tile([P, TBLK], F32,
                                              name=f"sg{b}_{e}_{i}", tag="sg")
                            nc.scalar.activation(
                                sg[:], g_ps[:],
                                mybir.ActivationFunctionType.Sigmoid)
                            t2 = sg_pool.tile([P, TBLK], F32,
                                              name=f"t2{b}_{e}_{i}", tag="t2")
                            nc.vector.tensor_mul(t2[:], sg[:], g_ps[:])
                            h_sb = h_pool.tile([P, TBLK], BF16,
                                               name=f"h{b}_{e}_{i}", tag="h")
                            nc.vector.tensor_mul(h_sb[:], t2[:], u_ps[:])
                            h_tiles.append(h_sb)

                        wd_tiles = []
                        for i in range(IC):
                            wd_sb = wd_pool.tile([P, D], BF16,
                                                 name=f"wd{b}_{e}_{i}", tag=f"wd{i}")
                            nc.gpsimd.dma_start(
                                out=wd_sb[:],
                                in_=wd_d[e, i * P:(i + 1) * P, :])
                            wd_tiles.append(wd_sb)

                        for t in range(TPB):
                            tci = b * TPB + t
                            wcol = w_signed[tci][:, e:e + 1]
                            for half in range(2):
                                eo_ps = eo_psum.tile(
                                    [P, D // 2], F32,
                                    name=f"eo{b}_{e}_{t}_{half}", tag="eo")
                                for n in range(2):
                                    nsl = slice(n * TBLK, (n + 1) * TBLK)
                                    for i in range(IC):
                                        nc.tensor.matmul(
                                            eo_ps[:, nsl],
                                            h_tiles[i][:, t * P:(t + 1) * P],
                                            wd_tiles[i][:,
                                                        half * (D // 2)
                                                        + n * TBLK:
                                                        half * (D // 2)
                                                        + (n + 1) * TBLK],
                                            start=(i == 0), stop=(i == IC - 1))
                                acc_sl = acc[t][:, half * (D // 2):
                                                (half + 1) * (D // 2)]
                                if e == 0:
                                    nc.vector.tensor_scalar_mul(
                                        acc_sl, eo_ps[:], wcol)
                                else:
                                    nc.vector.scalar_tensor_tensor(
                                        acc_sl, eo_ps[:], wcol, acc_sl,
                                        op0=au.mult, op1=au.add)

                    for t in range(TPB):
                        tci = b * TPB + t
                        nc.sync.dma_start(
                            out_d[tci * P:(tci + 1) * P, :], acc[t][:])
                        nc.scalar.activation(
                            dmy[:], acc[t][:],
                            mybir.ActivationFunctionType.Square,
                            accum_out=ss_sb[:, tci:tci + 1])

            nc.sync.dma_start(ss_d[:], ss_sb[:])

    nc.compile()
    return nc


def _get_nc():
    if "nc" not in _cached:
        _cached["nc"] = build()
    return _cached["nc"]


def kernel(x, Wr, Wg, Wu, Wd, _trace=False):
    x = np.ascontiguousarray(np.asarray(x, dtype=np.float32))
    Wr = np.ascontiguousarray(np.asarray(Wr, dtype=np.float32))
    Wg = np.ascontiguousarray(np.asarray(Wg, dtype=np.float32))
    Wu = np.ascontiguousarray(np.asarray(Wu, dtype=np.float32))
    Wd = np.ascontiguousarray(np.asarray(Wd, dtype=np.float32))

    x2 = x.reshape(TOKENS, D)
    nc = _get_nc()
    in_maps = []
    for c in range(N_CORES):
        in_maps.append({
            "x": x2[c * TC:(c + 1) * TC],
            "wr": Wr, "wg": Wg, "wu": Wu, "wd": Wd,
        })
    res = run_bass_kernel_spmd(nc, in_maps, core_ids=list(range(N_CORES)),
                               trace=_trace)
    out = np.concatenate([res.results[c]["out"] for c in range(N_CORES)],
                         axis=0).reshape(B, T, D)
    ss = sum(float(res.results[c]["ss"].sum(dtype=np.float64))
             for c in range(N_CORES))
    tension = np.float32(ss / (TOKENS * D))
    if _trace:
        kernel.last_results = res
    return out, tension
